# revision 54
# baseline (speedup 1.0000x reference)
"""Trainium2 Bass kernel for nn_H_ATT (GatedTrans pair-attention block).

Math (per example):
  HE = tanh(hist@W_hy+b_hy) * lrelu(hist@W_hg+b_hg)      [R, H]
  QE = tanh(ques@W_qy+b_qy) * lrelu(ques@W_qg+b_qg)      [R, H]
  num[q,h]  = sum_k QE[q,k]*W_att[k]*HE[h,k]
  den[q,h]  = sqrt(sum_k QE[q,k]^2 * HE[h,k]^2)
  s = num / max(den, eps)          (b_att cancels in softmax)
  att = causal_softmax(s)          (softmax*tril/renorm == masked softmax)
  feat = att @ hist                 [R, 2H]

Sharding: pure data parallel, 8 examples per core on 8 NeuronCores.

The embedding GEMMs dominate both PE time and HBM traffic; they run in
fp8(e4m3) with DoubleRow (weights pre-scaled by 256 on the host, descale
fused into the activation's scale argument). The score/att/feat path stays
bf16/f32. All DRAM inputs are host-packed so every DMA line is contiguous
per partition.
"""

import numpy as np
import ml_dtypes

import bass_rust
import concourse.bass as bass
import concourse.mybir as mybir
import concourse.tile as tile
from concourse.vector_clock import ScopedClock

# ---------------------------------------------------------------------------
# Workaround: this walrus build accepts only ONE semaphore wait on an SP
# Drain, but TileContext's tail drain carries one wait per live semaphore.
# Split them across a chain of drains.
# ---------------------------------------------------------------------------


def _patched_drain_and_barrier(self, tick_clock, wait_clock):
    nc = self.nc
    drain_inst = nc.sync.drain()
    wait_clock.add_sem_waits(
        drain_inst.ins, ScopedClock({None: tick_clock.global_clock})
    )
    waits = list(drain_inst.ins.sync_info.on_wait)
    if len(waits) > 1:
        drain_inst.ins.sync_info = bass_rust.SyncInfo(
            on_wait=waits[:1], on_update=list(drain_inst.ins.sync_info.on_update)
        )
        for i in range(1, len(waits)):
            extra = nc.sync.drain()
            extra.ins.sync_info = bass_rust.SyncInfo(
                on_wait=waits[i : i + 1], on_update=[]
            )
    nc.all_engine_barrier()
    assert self.sems is not None
    popped = nc._tile_sem_poison_stack.pop()
    assert popped is self._sem_poison
    # clear_and_free_semaphores without the ~6us RANGE_CLEAR InstISA or
    # the Q7 dma_reset (redundant with the SP drain chain above). A
    # sem-wr-imm costs ~0.5us apiece, but sem-sub-imm is a cheap posted
    # op (same class as the barrier's inc/dec) — subtract each sem's
    # statically-known final value (the drain chain's wait values).
    finals = {w.id: (w.ant_name, w.wait_value) for w in waits}
    sems = list(self.sems.allocated().values())
    engines = [nc.sync, nc.scalar, nc.vector, nc.tensor, nc.gpsimd]
    for i, s in enumerate(sems):
        inst = engines[i % len(engines)].sem_inc(s, 0)
        u = inst.ins.sync_info.on_update[0]
        if u.id in finals:
            upd = bass_rust.SyncUpdate(
                sync_type="semaphore", id=u.id, ant_name=u.ant_name,
                update_mode="sem-sub-imm", update_value=finals[u.id][1],
            )
        else:
            upd = bass_rust.SyncUpdate(
                sync_type="semaphore", id=u.id, ant_name=u.ant_name,
                update_mode="sem-wr-imm", update_value=0,
            )
        inst.ins.sync_info = bass_rust.SyncInfo(
            on_wait=list(inst.ins.sync_info.on_wait), on_update=[upd]
        )
    sem_nums = [s.num for s in sems]
    nc._state.prepend_free_semaphores(sem_nums)
    for poison_set in nc._tile_sem_poison_stack:
        poison_set.update(sem_nums)
    nc.all_engine_barrier()


tile.TileContext._drain_and_barrier = _patched_drain_and_barrier


def _cheapen_drain_chain(nc):
    """The teardown emits a chain of SP Drains (one sem wait each; see
    _patched_drain_and_barrier). A Drain costs ~0.5us; an EventSemaphore
    wait is ~0.1us. Keep only the final Drain, turn the rest into waits."""
    bb = nc.m.functions[0].blocks[-1]
    run = []
    for i, inst in enumerate(bb.instructions):
        si = inst.sync_info
        if (
            isinstance(inst, mybir.InstDrain)
            and inst.engine == mybir.EngineType.SP
            and si is not None
            and len(si.on_wait) == 1
            and len(si.on_update) == 0
        ):
            run.append(i)
        else:
            break
    for i in run[:-1]:
        old = bb.instructions[i]
        nop = mybir.InstEventSemaphore(
            name=f"I-drainwait-{i}", ins=[], outs=[]
        )
        nop.engine = old.engine
        nop.sync_info = old.sync_info
        bb.instructions[i] = nop
    # the all_engine_barrier emits a bare Pool (Q7) Drain per round that
    # costs ~6us; this kernel issues no gpsimd DMAs, so there is nothing
    # to drain there — drop them
    bb.instructions[:] = [
        inst
        for inst in bb.instructions
        if not (
            isinstance(inst, mybir.InstDrain)
            and inst.engine == mybir.EngineType.Pool
            and (
                inst.sync_info is None
                or (
                    len(inst.sync_info.on_wait) == 0
                    and len(inst.sync_info.on_update) == 0
                )
            )
        )
    ]
    # drop the SECOND all_engine_barrier entirely: the sem-clear nops are
    # posted ops on each engine's own stream, so they complete before the
    # engine halts; the next execution's runtime start gate re-syncs.
    # (barrier-1 must stay — clears have to follow the DMA drain chain.)
    last_clear = None
    for i, inst in enumerate(bb.instructions):
        si = inst.sync_info
        if si is None:
            continue
        for u in si.on_update:
            if u.update_mode in ("sem-sub-imm", "sem-wr-imm") and (
                "barrier" not in u.ant_name
            ):
                last_clear = i
    if last_clear is not None:
        del bb.instructions[last_clear + 1 :]


def _thin_pe_sem_updates(nc):
    """Every matmul carries a PE sem-inc, and the sequencer sync pipe
    processes them at ~115ns each — a large backlog that delays the
    teardown. Only accumulation-group-final (stop=True) matmuls are ever
    waited on; strip the rest and renumber every wait/final on that sem."""
    insts = [i for f in nc.m.functions for bb in f.blocks for i in bb.instructions]
    pe_sem = None
    updaters = []
    for inst in insts:
        si = inst.sync_info
        if si is None:
            continue
        for u in si.on_update:
            if u.ant_name.startswith("PE_") and u.update_mode == "sem-inc":
                pe_sem = u.id
                updaters.append(inst)
    if pe_sem is None:
        return
    kept = [
        isinstance(inst, mybir.InstMatmult) and bool(inst.stop_tensor_calc)
        for inst in updaters
    ]
    # any wait pointing at a non-kept updater must keep that updater
    for inst in insts:
        si = inst.sync_info
        if si is None:
            continue
        for w in si.on_wait:
            if w.id == pe_sem and 0 < w.wait_value <= len(kept):
                kept[w.wait_value - 1] = True
    prefix = []
    c = 0
    for k in kept:
        c += k
        prefix.append(c)
    total = c
    # strip updates from non-kept matmuls
    for inst, k in zip(updaters, kept):
        if k:
            continue
        si = inst.sync_info
        inst.sync_info = bass_rust.SyncInfo(
            on_wait=list(si.on_wait),
            on_update=[u for u in si.on_update if u.id != pe_sem],
        )
    # renumber waits and the teardown's sem-sub final
    for inst in insts:
        si = inst.sync_info
        if si is None:
            continue
        dirty = False
        ws = []
        for w in si.on_wait:
            if w.id == pe_sem and 0 < w.wait_value <= len(kept):
                ws.append(
                    bass_rust.SyncWait(
                        sync_type=w.sync_type, id=w.id, ant_name=w.ant_name,
                        wait_mode=w.wait_mode,
                        wait_value=prefix[w.wait_value - 1],
                    )
                )
                dirty = True
            else:
                ws.append(w)
        us = []
        for u in si.on_update:
            if u.id == pe_sem and u.update_mode == "sem-sub-imm":
                us.append(
                    bass_rust.SyncUpdate(
                        sync_type=u.sync_type, id=u.id, ant_name=u.ant_name,
                        update_mode="sem-sub-imm", update_value=total,
                    )
                )
                dirty = True
            else:
                us.append(u)
        if dirty:
            inst.sync_info = bass_rust.SyncInfo(on_wait=ws, on_update=us)


def _split_multi_waits(nc):
    """This walrus build accepts at most one semaphore wait per instruction.
    Hoist extra waits onto standalone EventSemaphore instructions inserted
    just before the owning instruction in the same engine's stream."""
    uid = [0]
    for f in nc.m.functions:
        for bb in f.blocks:
            out = []
            for inst in bb.instructions:
                si = inst.sync_info
                if si is not None and len(si.on_wait) > 1:
                    waits = list(si.on_wait)
                    for w in waits[:-1]:
                        nop = mybir.InstEventSemaphore(
                            name=f"I-waitsplit-{uid[0]}", ins=[], outs=[]
                        )
                        uid[0] += 1
                        nop.engine = inst.engine
                        nop.sync_info = bass_rust.SyncInfo(
                            on_wait=[w], on_update=[]
                        )
                        out.append(nop)
                    inst.sync_info = bass_rust.SyncInfo(
                        on_wait=[waits[-1]], on_update=list(si.on_update)
                    )
                out.append(inst)
            bb.instructions[:] = out

# ---------------------------------------------------------------------------

B, R, H, IN = 64, 32, 1024, 2048
NCORES = 8
BL = B // NCORES  # examples per core
BR = BL * R  # 256 rows per core
MC = H // 128  # 8 h chunks
NEG = -1.0e30
WSCALE = 256.0  # fp8 weight pre-scale

F32 = mybir.dt.float32
BF16 = mybir.dt.bfloat16
E4 = mybir.dt.float8e4

ACT = mybir.ActivationFunctionType
DR = mybir.MatmulPerfMode.DoubleRow


def build_program(fp8=True):
    """Per-core Bass program. fp8: embedding GEMMs in e4m3 + DoubleRow;
    else bf16 regular matmuls. Everything downstream is identical."""
    xdt = E4 if fp8 else BF16
    KC = 8 if fp8 else 16  # contraction chunks ([128,2] pairs when fp8)
    kshape = [KC, 2, 128] if fp8 else [KC, 128]
    ascale = 1.0 / WSCALE if fp8 else 1.0

    nc = bass.Bass()
    # activations: [128, KC(,2), BR], partition-major contiguous
    qt_d = nc.dram_tensor("qt", [128] + kshape[:-1] + [BR], xdt, kind="ExternalInput")
    ht_d = nc.dram_tensor("ht", [128] + kshape[:-1] + [BR], xdt, kind="ExternalInput")
    # weights: [MC, 128, 2branch, KC(,2), 128]
    wh_d = nc.dram_tensor("wh", [MC, 128, 2] + kshape, xdt, kind="ExternalInput")
    wq_d = nc.dram_tensor("wq", [MC, 128, 2] + kshape, xdt, kind="ExternalInput")
    hn_d = nc.dram_tensor("hn", [128, 2, IN], BF16, kind="ExternalInput")
    # consts packed: [bqy, bqg, bhy, bhg, watt] along dim1
    cb_d = nc.dram_tensor("cb", [128, 5, MC], F32, kind="ExternalInput")
    # transposed causal/block mask [h, q]
    mi_d = nc.dram_tensor("mi", [128, 128], F32, kind="ExternalInput")
    feat_d = nc.dram_tensor("feat", [2, 128, IN], BF16, kind="ExternalOutput")

    with tile.TileContext(nc) as tc:
        with (
            tc.tile_pool(name="sb", bufs=1) as sb,
            tc.tile_pool(name="wts", bufs=8) as wts,
            tc.tile_pool(name="tmp", bufs=3) as tmp,
        ):
            # acts + consts on the ACT hwdge queue (independent of the
            # weight queue so weight-buffer waits never delay them).
            # qt split in two tiles so the first matmuls start earlier.
            KH = KC // 2
            qta = sb.tile([128, KH] + kshape[1:-1] + [BR], xdt, tag="qta")
            nc.scalar.dma_start(qta[:], qt_d[:, :KH])
            qtb = sb.tile([128, KH] + kshape[1:-1] + [BR], xdt, tag="qtb")
            nc.scalar.dma_start(qtb[:], qt_d[:, KH:])

            def qt(k):
                return qta[:, k] if k < KH else qtb[:, k - KH]

            # tiny consts next (first activations need the biases), then
            # the bulk hist loads (not needed until the hist phase / tail)
            cb = sb.tile([128, 5, MC], F32, tag="cb")
            nc.scalar.dma_start(cb[:], cb_d[:])
            bsb = {
                "bqy": cb[:, 0], "bqg": cb[:, 1],
                "bhy": cb[:, 2], "bhg": cb[:, 3],
            }
            watt = cb[:, 4]

            ht_t = sb.tile([128] + kshape[:-1] + [BR], xdt, tag="ht")
            nc.scalar.dma_start(ht_t[:], ht_d[:])

            def ht(k):
                return ht_t[:, k]

            mask_t = sb.tile([128, 128], F32, tag="mask")
            mask = mask_t[:]
            hn = sb.tile([128, 2, IN], BF16, tag="hn")
            ones = sb.tile([128, 1], BF16, tag="ones")
            nc.vector.memset(ones[:], 1.0)

            he = sb.tile([128, MC, BR], BF16, tag="he")
            he2 = sb.tile([128, MC, BR], BF16, tag="he2")
            qew = sb.tile([128, MC, BR], BF16, tag="qew")
            qe2 = sb.tile([128, MC, BR], BF16, tag="qe2")

            with (
                tc.tile_pool(name="pse", bufs=2, space="PSUM") as pse,
                tc.tile_pool(name="psnd", bufs=1, space="PSUM") as psnd,
            ):
                num_ps = [
                    psnd.tile([128, 128], F32, name=f"num{g}", tag=f"num{g}")
                    for g in range(2)
                ]
                den_ps = [
                    psnd.tile([128, 128], F32, name=f"den{g}", tag=f"den{g}")
                    for g in range(2)
                ]
                num_ps = [t[:] for t in num_ps]
                den_ps = [t[:] for t in den_ps]

                # weight DMAs pre-issued on the sync queue: wh0 interleaved
                # before the ques tail so the hist phase isn't gated on it
                # (the queue delivers ~one 512KB tile per 1.6us, which only
                # just keeps pace with PE's ~1.75us per m-chunk)
                worder = (
                    [("q", m) for m in range(6)]
                    + [("h", 0), ("q", 6), ("q", 7)]
                    + [("h", m) for m in range(1, MC)]
                )
                wtiles = {}
                for i, (t, m) in enumerate(worder):
                    wt = wts.tile([128, 2] + kshape, xdt, tag="wt")
                    src_d = (wq_d if t == "q" else wh_d)[m]
                    if i == 0:
                        # split the very first weight tile by branch so the
                        # first psy matmuls start half a transfer earlier
                        nc.sync.dma_start(wt[:, 0], src_d[:, 0])
                        nc.sync.dma_start(wt[:, 1], src_d[:, 1])
                    else:
                        nc.sync.dma_start(wt[:], src_d)
                    wtiles[(t, m)] = wt
                # tail-only loads at the back of the weight queue: keeps
                # their triggers off the ACT sequencer entirely
                nc.sync.dma_start(mask_t[:], mi_d[:])
                nc.sync.dma_start(hn[:], hn_d[:])

                def gated(xt, tkey, by, bg, m):
                    """Embedding GEMM pair + activations; returns ty, tg f32."""
                    wt = wtiles[(tkey, m)]
                    psy = pse.tile([128, BR], F32, tag="psy")
                    psg = pse.tile([128, BR], F32, tag="psg")
                    for br, ps in ((0, psy), (1, psg)):
                        for k in range(KC):
                            if fp8:
                                nc.tensor.matmul(
                                    ps[:], wt[:, br, k], xt(k),
                                    start=(k == 0), stop=(k == KC - 1),
                                    perf_mode=DR,
                                )
                            else:
                                nc.tensor.matmul(
                                    ps[:], wt[:, br, k], xt(k),
                                    start=(k == 0), stop=(k == KC - 1),
                                )
                    ty = tmp.tile([128, BR], F32, tag="ty")
                    nc.scalar.activation(
                        ty[:], psy[:], ACT.Tanh,
                        bias=by[:, m : m + 1], scale=ascale,
                    )
                    tg = tmp.tile([128, BR], F32, tag="tg")
                    nc.scalar.activation(
                        tg[:], psg[:], ACT.Lrelu,
                        bias=bg[:, m : m + 1], scale=ascale, alpha=0.01,
                    )
                    return ty, tg

                # ques embeddings
                for m in range(MC):
                    ty, tg = gated(qt, "q", bsb["bqy"], bsb["bqg"], m)
                    nc.vector.scalar_tensor_tensor(
                        qew[:, m, :], ty[:], watt[:, m : m + 1], tg[:],
                        op0=mybir.AluOpType.mult, op1=mybir.AluOpType.mult,
                    )
                    qe = tmp.tile([128, BR], F32, tag="qe")
                    nc.vector.tensor_mul(qe[:], ty[:], tg[:])
                    nc.gpsimd.tensor_mul(qe2[:, m, :], qe[:], qe[:])

                # hist embeddings + num/den accumulation per chunk
                for m in range(MC):
                    ty, tg = gated(ht, "h", bsb["bhy"], bsb["bhg"], m)
                    nc.vector.tensor_mul(he[:, m, :], ty[:], tg[:])
                    nc.gpsimd.tensor_mul(he2[:, m, :], he[:, m, :], he[:, m, :])
                    for g in range(2):
                        sl = slice(128 * g, 128 * (g + 1))
                        # transposed: stationary he, moving qew -> num^T[h,q];
                        # the tail then needs no PE transpose at all.
                        # den first on the last chunk: the sqrt chain is the
                        # tail's critical path
                        mm = [
                            (num_ps[g], he, qew),
                            (den_ps[g], he2, qe2),
                        ]
                        if m == MC - 1:
                            mm.reverse()
                        for ps, a, b in mm:
                            nc.tensor.matmul(
                                ps, a[:, m, sl], b[:, m, sl],
                                start=(m == 0), stop=(m == MC - 1),
                            )

                # transposed scores + softmax numerator while num/den PSUM is
                # still mapped; exp writes att^T directly as bf16 (the feat
                # GEMM stationary); both sqrts back-to-back keep ACT busy
                # while DVE runs the reciprocal/mask chain
                atts = []
                sds = []
                for g in range(2):
                    sd = tmp.tile([128, 128], F32, tag="sd")
                    nc.scalar.activation(sd[:], den_ps[g], ACT.Sqrt)
                    sds.append(sd)
                for g in range(2):
                    rd = tmp.tile([128, 128], F32, tag="rd")
                    nc.vector.reciprocal(rd[:], sds[g][:])
                    s = tmp.tile([128, 128], F32, tag="s")
                    nc.vector.tensor_mul(s[:], num_ps[g], rd[:])
                    nc.vector.tensor_add(s[:], s[:], mask)
                    atb = sb.tile([128, 128], BF16, name=f"atb{g}", tag=f"atb{g}")
                    nc.scalar.activation(atb[:], s[:], ACT.Exp)
                    atts.append(atb)

            # feat GEMMs; row-sums via a ones-vector matmul on the same
            # stationary att^T, renorm folded into the PSUM->SBUF copies
            with (
                tc.tile_pool(name="psf", bufs=2, space="PSUM") as psf,
                tc.tile_pool(name="psr", bufs=1, space="PSUM") as psr,
            ):
                for g in range(2):
                    atb = atts[g]
                    rsp = psr.tile([128, 1], F32, name=f"rs{g}", tag=f"rs{g}")
                    nc.tensor.matmul(rsp[:], atb[:], ones[:], start=True, stop=True)
                    rrs = sb.tile([128, 1], F32, name=f"rrs{g}", tag=f"rrs{g}")
                    nc.vector.reciprocal(rrs[:], rsp[:])
                    for h in range(2):
                        # one 2-bank PSUM tile per half: two matmul groups
                        # (one full bank each). The two 512-wide scaled
                        # copies go to SEPARATE SBUF tiles so DVE and ACT
                        # truly run concurrently (same-tile writers appear
                        # to serialize), each with its own DMA on the
                        # tail-idle sync queue.
                        fps = psf.tile([128, 1024], F32, tag="fps")
                        for c in range(2):
                            cs = slice(1024 * h + 512 * c, 1024 * h + 512 * (c + 1))
                            nc.tensor.matmul(
                                fps[:, 512 * c : 512 * (c + 1)], atb[:],
                                hn[:, g, cs], start=True, stop=True,
                            )
                        flo = sb.tile(
                            [128, 512], BF16, name=f"fl{g}{h}", tag=f"fl{g}{h}"
                        )
                        fhi = sb.tile(
                            [128, 512], BF16, name=f"fh{g}{h}", tag=f"fh{g}{h}"
                        )
                        nc.vector.tensor_scalar_mul(flo[:], fps[:, :512], rrs[:])
                        nc.scalar.activation(
                            fhi[:], fps[:, 512:], ACT.Copy, scale=rrs[:]
                        )
                        lo = slice(1024 * h, 1024 * h + 512)
                        hi = slice(1024 * h + 512, 1024 * (h + 1))
                        nc.sync.dma_start(feat_d[g, :, lo], flo[:])
                        nc.sync.dma_start(feat_d[g, :, hi], fhi[:])

    _cheapen_drain_chain(nc)
    _split_multi_waits(nc)
    return nc


# ---------------------------------------------------------------------------
# Host side
# ---------------------------------------------------------------------------

_PROG_CACHE = {}


def _get_prog(fp8):
    if fp8 not in _PROG_CACHE:
        _PROG_CACHE[fp8] = build_program(fp8)
    return _PROG_CACHE[fp8]


def _pack_acts(x, fp8):
    """[BR, IN] -> [128, KC(,2), BR] with k_eff = 256k+128j+p (fp8) or
    128k+p (bf16); contiguous per partition."""
    xt = np.ascontiguousarray(x.T)  # [IN, BR]
    if fp8:
        a = xt.reshape(8, 2, 128, BR).transpose(2, 0, 1, 3)
        return np.ascontiguousarray(np.clip(a, -240, 240)).astype(
            ml_dtypes.float8_e4m3
        )
    a = xt.reshape(16, 128, BR).transpose(1, 0, 2)
    return np.ascontiguousarray(a).astype(ml_dtypes.bfloat16)


def _pack_w(Wy, Wg, fp8):
    """2x[IN, H] -> [MC, 128, 2, KC(,2), 128], scaled for fp8."""
    def one(W):
        if fp8:
            # [k8, j2, p128, m8, h128] -> [m, p, k, j, h]
            a = W.reshape(8, 2, 128, MC, 128).transpose(3, 2, 0, 1, 4)
            a = np.clip(a * WSCALE, -240, 240)
            return a.astype(ml_dtypes.float8_e4m3)
        a = W.reshape(16, 128, MC, 128).transpose(2, 1, 0, 3)
        return a.astype(ml_dtypes.bfloat16)

    y, g = one(Wy), one(Wg)
    return np.ascontiguousarray(np.stack([y, g], axis=2))


def _prep_shared(W_hy, b_hy, W_hg, b_hg, W_qy, b_qy, W_qg, b_qg, W_att, fp8):
    def bvec(b):
        return np.ascontiguousarray(b.reshape(MC, 128).T).astype(np.float32)

    # transposed block-diagonal causal mask, indexed [h_row, q_row]:
    # 0 where (same example AND h_round <= q_round), NEG elsewhere
    r = np.arange(128)
    same_ex = r[:, None] // 32 == r[None, :] // 32
    causal_t = (r[:, None] % 32) <= (r[None, :] % 32)
    mask_t = np.where(same_ex & causal_t, 0.0, NEG).astype(np.float32)

    cb = np.stack(
        [bvec(b_qy), bvec(b_qg), bvec(b_hy), bvec(b_hg), bvec(W_att)], axis=1
    )
    return {
        "wh": _pack_w(W_hy, W_hg, fp8),
        "wq": _pack_w(W_qy, W_qg, fp8),
        "cb": np.ascontiguousarray(cb),
        "mi": np.ascontiguousarray(mask_t),
    }


def kernel(
    hist, ques, W_hy, b_hy, W_hg, b_hg, W_qy, b_qy, W_qg, b_qg, W_att, b_att,
    mode="fp8", trace=False,
):
    from concourse.bass_utils import run_bass_kernel_spmd

    fp8 = mode == "fp8"
    hist = np.asarray(hist, np.float32)
    ques = np.asarray(ques, np.float32)
    nc = _get_prog(fp8)
    shared = _prep_shared(
        np.asarray(W_hy, np.float32), np.asarray(b_hy, np.float32),
        np.asarray(W_hg, np.float32), np.asarray(b_hg, np.float32),
        np.asarray(W_qy, np.float32), np.asarray(b_qy, np.float32),
        np.asarray(W_qg, np.float32), np.asarray(b_qg, np.float32),
        np.asarray(W_att, np.float32), fp8,
    )
    in_maps = []
    for c in range(NCORES):
        hs = hist[c * BL : (c + 1) * BL].reshape(BR, IN)
        qs = ques[c * BL : (c + 1) * BL].reshape(BR, IN)
        im = dict(shared)
        im["qt"] = _pack_acts(qs, fp8)
        im["ht"] = _pack_acts(hs, fp8)
        im["hn"] = np.ascontiguousarray(
            hs.reshape(2, 128, IN).transpose(1, 0, 2)
        ).astype(ml_dtypes.bfloat16)
        in_maps.append(im)

    res = run_bass_kernel_spmd(
        nc, in_maps, core_ids=list(range(NCORES)), trace=trace
    )
    feat = np.concatenate(
        [
            r["feat"].astype(np.float32).reshape(BL, R, IN)
            for r in res.results
        ],
        axis=0,
    )
    if trace:
        return feat, res
    return feat


# revision 55
# speedup vs baseline: 1.0008x; 1.0008x over previous
"""Trainium2 Bass kernel for nn_H_ATT (GatedTrans pair-attention block).

Math (per example):
  HE = tanh(hist@W_hy+b_hy) * lrelu(hist@W_hg+b_hg)      [R, H]
  QE = tanh(ques@W_qy+b_qy) * lrelu(ques@W_qg+b_qg)      [R, H]
  num[q,h]  = sum_k QE[q,k]*W_att[k]*HE[h,k]
  den[q,h]  = sqrt(sum_k QE[q,k]^2 * HE[h,k]^2)
  s = num / max(den, eps)          (b_att cancels in softmax)
  att = causal_softmax(s)          (softmax*tril/renorm == masked softmax)
  feat = att @ hist                 [R, 2H]

Sharding: pure data parallel, 8 examples per core on 8 NeuronCores.

The embedding GEMMs dominate both PE time and HBM traffic; they run in
fp8(e4m3) with DoubleRow (weights pre-scaled by 256 on the host, descale
fused into the activation's scale argument). The score/att/feat path stays
bf16/f32. All DRAM inputs are host-packed so every DMA line is contiguous
per partition.
"""

import numpy as np
import ml_dtypes

import bass_rust
import concourse.bass as bass
import concourse.mybir as mybir
import concourse.tile as tile
from concourse.vector_clock import ScopedClock

# ---------------------------------------------------------------------------
# Workaround: this walrus build accepts only ONE semaphore wait on an SP
# Drain, but TileContext's tail drain carries one wait per live semaphore.
# Split them across a chain of drains.
# ---------------------------------------------------------------------------


def _patched_drain_and_barrier(self, tick_clock, wait_clock):
    nc = self.nc
    drain_inst = nc.sync.drain()
    wait_clock.add_sem_waits(
        drain_inst.ins, ScopedClock({None: tick_clock.global_clock})
    )
    waits = list(drain_inst.ins.sync_info.on_wait)
    if len(waits) > 1:
        drain_inst.ins.sync_info = bass_rust.SyncInfo(
            on_wait=waits[:1], on_update=list(drain_inst.ins.sync_info.on_update)
        )
        for i in range(1, len(waits)):
            extra = nc.sync.drain()
            extra.ins.sync_info = bass_rust.SyncInfo(
                on_wait=waits[i : i + 1], on_update=[]
            )
    nc.all_engine_barrier()
    assert self.sems is not None
    popped = nc._tile_sem_poison_stack.pop()
    assert popped is self._sem_poison
    # clear_and_free_semaphores without the ~6us RANGE_CLEAR InstISA or
    # the Q7 dma_reset (redundant with the SP drain chain above). A
    # sem-wr-imm costs ~0.5us apiece, but sem-sub-imm is a cheap posted
    # op (same class as the barrier's inc/dec) — subtract each sem's
    # statically-known final value (the drain chain's wait values).
    finals = {w.id: (w.ant_name, w.wait_value) for w in waits}
    sems = list(self.sems.allocated().values())
    engines = [nc.sync, nc.scalar, nc.vector, nc.tensor, nc.gpsimd]
    for i, s in enumerate(sems):
        inst = engines[i % len(engines)].sem_inc(s, 0)
        u = inst.ins.sync_info.on_update[0]
        if u.id in finals:
            upd = bass_rust.SyncUpdate(
                sync_type="semaphore", id=u.id, ant_name=u.ant_name,
                update_mode="sem-sub-imm", update_value=finals[u.id][1],
            )
        else:
            upd = bass_rust.SyncUpdate(
                sync_type="semaphore", id=u.id, ant_name=u.ant_name,
                update_mode="sem-wr-imm", update_value=0,
            )
        inst.ins.sync_info = bass_rust.SyncInfo(
            on_wait=list(inst.ins.sync_info.on_wait), on_update=[upd]
        )
    sem_nums = [s.num for s in sems]
    nc._state.prepend_free_semaphores(sem_nums)
    for poison_set in nc._tile_sem_poison_stack:
        poison_set.update(sem_nums)
    nc.all_engine_barrier()


tile.TileContext._drain_and_barrier = _patched_drain_and_barrier


def _cheapen_drain_chain(nc):
    """The teardown emits a chain of SP Drains (one sem wait each; see
    _patched_drain_and_barrier). A Drain costs ~0.5us; an EventSemaphore
    wait is ~0.1us. Keep only the final Drain, turn the rest into waits."""
    bb = nc.m.functions[0].blocks[-1]
    run = []
    for i, inst in enumerate(bb.instructions):
        si = inst.sync_info
        if (
            isinstance(inst, mybir.InstDrain)
            and inst.engine == mybir.EngineType.SP
            and si is not None
            and len(si.on_wait) == 1
            and len(si.on_update) == 0
        ):
            run.append(i)
        else:
            break
    for i in run[:-1]:
        old = bb.instructions[i]
        nop = mybir.InstEventSemaphore(
            name=f"I-drainwait-{i}", ins=[], outs=[]
        )
        nop.engine = old.engine
        nop.sync_info = old.sync_info
        bb.instructions[i] = nop
    # the all_engine_barrier emits a bare Pool (Q7) Drain per round that
    # costs ~6us; this kernel issues no gpsimd DMAs, so there is nothing
    # to drain there — drop them
    bb.instructions[:] = [
        inst
        for inst in bb.instructions
        if not (
            isinstance(inst, mybir.InstDrain)
            and inst.engine == mybir.EngineType.Pool
            and (
                inst.sync_info is None
                or (
                    len(inst.sync_info.on_wait) == 0
                    and len(inst.sync_info.on_update) == 0
                )
            )
        )
    ]
    # drop the SECOND all_engine_barrier entirely: the sem-clear nops are
    # posted ops on each engine's own stream, so they complete before the
    # engine halts; the next execution's runtime start gate re-syncs.
    # (barrier-1 must stay — clears have to follow the DMA drain chain.)
    last_clear = None
    for i, inst in enumerate(bb.instructions):
        si = inst.sync_info
        if si is None:
            continue
        for u in si.on_update:
            if u.update_mode in ("sem-sub-imm", "sem-wr-imm") and (
                "barrier" not in u.ant_name
            ):
                last_clear = i
    if last_clear is not None:
        del bb.instructions[last_clear + 1 :]


def _thin_pe_sem_updates(nc):
    """Every matmul carries a PE sem-inc, and the sequencer sync pipe
    processes them at ~115ns each — a large backlog that delays the
    teardown. Only accumulation-group-final (stop=True) matmuls are ever
    waited on; strip the rest and renumber every wait/final on that sem."""
    insts = [i for f in nc.m.functions for bb in f.blocks for i in bb.instructions]
    pe_sem = None
    updaters = []
    for inst in insts:
        si = inst.sync_info
        if si is None:
            continue
        for u in si.on_update:
            if u.ant_name.startswith("PE_") and u.update_mode == "sem-inc":
                pe_sem = u.id
                updaters.append(inst)
    if pe_sem is None:
        return
    kept = [
        isinstance(inst, mybir.InstMatmult) and bool(inst.stop_tensor_calc)
        for inst in updaters
    ]
    # any wait pointing at a non-kept updater must keep that updater
    for inst in insts:
        si = inst.sync_info
        if si is None:
            continue
        for w in si.on_wait:
            if w.id == pe_sem and 0 < w.wait_value <= len(kept):
                kept[w.wait_value - 1] = True
    prefix = []
    c = 0
    for k in kept:
        c += k
        prefix.append(c)
    total = c
    # strip updates from non-kept matmuls
    for inst, k in zip(updaters, kept):
        if k:
            continue
        si = inst.sync_info
        inst.sync_info = bass_rust.SyncInfo(
            on_wait=list(si.on_wait),
            on_update=[u for u in si.on_update if u.id != pe_sem],
        )
    # renumber waits and the teardown's sem-sub final
    for inst in insts:
        si = inst.sync_info
        if si is None:
            continue
        dirty = False
        ws = []
        for w in si.on_wait:
            if w.id == pe_sem and 0 < w.wait_value <= len(kept):
                ws.append(
                    bass_rust.SyncWait(
                        sync_type=w.sync_type, id=w.id, ant_name=w.ant_name,
                        wait_mode=w.wait_mode,
                        wait_value=prefix[w.wait_value - 1],
                    )
                )
                dirty = True
            else:
                ws.append(w)
        us = []
        for u in si.on_update:
            if u.id == pe_sem and u.update_mode == "sem-sub-imm":
                us.append(
                    bass_rust.SyncUpdate(
                        sync_type=u.sync_type, id=u.id, ant_name=u.ant_name,
                        update_mode="sem-sub-imm", update_value=total,
                    )
                )
                dirty = True
            else:
                us.append(u)
        if dirty:
            inst.sync_info = bass_rust.SyncInfo(on_wait=ws, on_update=us)


def _split_multi_waits(nc):
    """This walrus build accepts at most one semaphore wait per instruction.
    Hoist extra waits onto standalone EventSemaphore instructions inserted
    just before the owning instruction in the same engine's stream."""
    uid = [0]
    for f in nc.m.functions:
        for bb in f.blocks:
            out = []
            for inst in bb.instructions:
                si = inst.sync_info
                if si is not None and len(si.on_wait) > 1:
                    waits = list(si.on_wait)
                    for w in waits[:-1]:
                        nop = mybir.InstEventSemaphore(
                            name=f"I-waitsplit-{uid[0]}", ins=[], outs=[]
                        )
                        uid[0] += 1
                        nop.engine = inst.engine
                        nop.sync_info = bass_rust.SyncInfo(
                            on_wait=[w], on_update=[]
                        )
                        out.append(nop)
                    inst.sync_info = bass_rust.SyncInfo(
                        on_wait=[waits[-1]], on_update=list(si.on_update)
                    )
                out.append(inst)
            bb.instructions[:] = out

# ---------------------------------------------------------------------------

B, R, H, IN = 64, 32, 1024, 2048
NCORES = 8
BL = B // NCORES  # examples per core
BR = BL * R  # 256 rows per core
MC = H // 128  # 8 h chunks
NEG = -1.0e30
WSCALE = 256.0  # fp8 weight pre-scale

F32 = mybir.dt.float32
BF16 = mybir.dt.bfloat16
E4 = mybir.dt.float8e4

ACT = mybir.ActivationFunctionType
DR = mybir.MatmulPerfMode.DoubleRow


def build_program(fp8=True):
    """Per-core Bass program. fp8: embedding GEMMs in e4m3 + DoubleRow;
    else bf16 regular matmuls. Everything downstream is identical."""
    xdt = E4 if fp8 else BF16
    KC = 8 if fp8 else 16  # contraction chunks ([128,2] pairs when fp8)
    kshape = [KC, 2, 128] if fp8 else [KC, 128]
    ascale = 1.0 / WSCALE if fp8 else 1.0

    nc = bass.Bass()
    # activations: [128, KC(,2), BR], partition-major contiguous
    qt_d = nc.dram_tensor("qt", [128] + kshape[:-1] + [BR], xdt, kind="ExternalInput")
    ht_d = nc.dram_tensor("ht", [128] + kshape[:-1] + [BR], xdt, kind="ExternalInput")
    # weights: [MC, 128, 2branch, KC(,2), 128]
    wh_d = nc.dram_tensor("wh", [MC, 128, 2] + kshape, xdt, kind="ExternalInput")
    wq_d = nc.dram_tensor("wq", [MC, 128, 2] + kshape, xdt, kind="ExternalInput")
    hn_d = nc.dram_tensor("hn", [128, 2, IN], BF16, kind="ExternalInput")
    # consts packed: [bqy, bqg, bhy, bhg, watt] along dim1
    cb_d = nc.dram_tensor("cb", [128, 5, MC], F32, kind="ExternalInput")
    # transposed causal/block mask [h, q]
    mi_d = nc.dram_tensor("mi", [128, 128], F32, kind="ExternalInput")
    feat_d = nc.dram_tensor("feat", [2, 128, IN], BF16, kind="ExternalOutput")

    with tile.TileContext(nc) as tc:
        with (
            tc.tile_pool(name="sb", bufs=1) as sb,
            tc.tile_pool(name="wts", bufs=8) as wts,
            tc.tile_pool(name="tmp", bufs=3) as tmp,
        ):
            # acts + consts on the ACT hwdge queue (independent of the
            # weight queue so weight-buffer waits never delay them).
            # qt split in two tiles so the first matmuls start earlier.
            KH = KC // 2
            qta = sb.tile([128, KH] + kshape[1:-1] + [BR], xdt, tag="qta")
            nc.scalar.dma_start(qta[:], qt_d[:, :KH])
            qtb = sb.tile([128, KH] + kshape[1:-1] + [BR], xdt, tag="qtb")
            nc.scalar.dma_start(qtb[:], qt_d[:, KH:])

            def qt(k):
                return qta[:, k] if k < KH else qtb[:, k - KH]

            # tiny consts next (first activations need the biases), then
            # the bulk hist loads (not needed until the hist phase / tail)
            cb = sb.tile([128, 5, MC], F32, tag="cb")
            nc.scalar.dma_start(cb[:], cb_d[:])
            bsb = {
                "bqy": cb[:, 0], "bqg": cb[:, 1],
                "bhy": cb[:, 2], "bhg": cb[:, 3],
            }
            watt = cb[:, 4]

            ht_t = sb.tile([128] + kshape[:-1] + [BR], xdt, tag="ht")
            nc.scalar.dma_start(ht_t[:], ht_d[:])

            def ht(k):
                return ht_t[:, k]

            mask_t = sb.tile([128, 128], F32, tag="mask")
            mask = mask_t[:]
            hn = sb.tile([128, 2, IN], BF16, tag="hn")
            ones = sb.tile([128, 1], BF16, tag="ones")
            nc.vector.memset(ones[:], 1.0)

            he = sb.tile([128, MC, BR], BF16, tag="he")
            he2 = sb.tile([128, MC, BR], BF16, tag="he2")
            qew = sb.tile([128, MC, BR], BF16, tag="qew")
            qe2 = sb.tile([128, MC, BR], BF16, tag="qe2")

            with (
                tc.tile_pool(name="pse", bufs=2, space="PSUM") as pse,
                tc.tile_pool(name="psnd", bufs=1, space="PSUM") as psnd,
            ):
                num_ps = [
                    psnd.tile([128, 128], F32, name=f"num{g}", tag=f"num{g}")
                    for g in range(2)
                ]
                den_ps = [
                    psnd.tile([128, 128], F32, name=f"den{g}", tag=f"den{g}")
                    for g in range(2)
                ]
                num_ps = [t[:] for t in num_ps]
                den_ps = [t[:] for t in den_ps]

                # weight DMAs pre-issued on the sync queue: wh0 interleaved
                # before the ques tail so the hist phase isn't gated on it
                # (the queue delivers ~one 512KB tile per 1.6us, which only
                # just keeps pace with PE's ~1.75us per m-chunk)
                worder = (
                    [("q", m) for m in range(6)]
                    + [("h", 0), ("q", 6), ("q", 7)]
                    + [("h", m) for m in range(1, MC)]
                )
                wtiles = {}
                for i, (t, m) in enumerate(worder):
                    wt = wts.tile([128, 2] + kshape, xdt, tag="wt")
                    src_d = (wq_d if t == "q" else wh_d)[m]
                    if i == 0:
                        # split the very first weight tile by branch so the
                        # first psy matmuls start half a transfer earlier
                        nc.sync.dma_start(wt[:, 0], src_d[:, 0])
                        nc.sync.dma_start(wt[:, 1], src_d[:, 1])
                    else:
                        nc.sync.dma_start(wt[:], src_d)
                    wtiles[(t, m)] = wt
                # tail-only loads at the back of the weight queue: keeps
                # their triggers off the ACT sequencer entirely
                nc.sync.dma_start(mask_t[:], mi_d[:])
                nc.sync.dma_start(hn[:], hn_d[:])

                def gated(xt, tkey, by, bg, m):
                    """Embedding GEMM pair + activations; returns ty, tg f32."""
                    wt = wtiles[(tkey, m)]
                    psy = pse.tile([128, BR], F32, tag="psy")
                    psg = pse.tile([128, BR], F32, tag="psg")
                    for br, ps in ((0, psy), (1, psg)):
                        for k in range(KC):
                            if fp8:
                                nc.tensor.matmul(
                                    ps[:], wt[:, br, k], xt(k),
                                    start=(k == 0), stop=(k == KC - 1),
                                    perf_mode=DR,
                                )
                            else:
                                nc.tensor.matmul(
                                    ps[:], wt[:, br, k], xt(k),
                                    start=(k == 0), stop=(k == KC - 1),
                                )
                    ty = tmp.tile([128, BR], F32, tag="ty")
                    nc.scalar.activation(
                        ty[:], psy[:], ACT.Tanh,
                        bias=by[:, m : m + 1], scale=ascale,
                    )
                    tg = tmp.tile([128, BR], F32, tag="tg")
                    nc.scalar.activation(
                        tg[:], psg[:], ACT.Lrelu,
                        bias=bg[:, m : m + 1], scale=ascale, alpha=0.01,
                    )
                    return ty, tg

                # ques embeddings
                for m in range(MC):
                    ty, tg = gated(qt, "q", bsb["bqy"], bsb["bqg"], m)
                    nc.vector.scalar_tensor_tensor(
                        qew[:, m, :], ty[:], watt[:, m : m + 1], tg[:],
                        op0=mybir.AluOpType.mult, op1=mybir.AluOpType.mult,
                    )
                    qe = tmp.tile([128, BR], F32, tag="qe")
                    nc.vector.tensor_mul(qe[:], ty[:], tg[:])
                    nc.gpsimd.tensor_mul(qe2[:, m, :], qe[:], qe[:])

                # hist embeddings + num/den accumulation per chunk
                for m in range(MC):
                    ty, tg = gated(ht, "h", bsb["bhy"], bsb["bhg"], m)
                    nc.vector.tensor_mul(he[:, m, :], ty[:], tg[:])
                    nc.gpsimd.tensor_mul(he2[:, m, :], he[:, m, :], he[:, m, :])
                    for g in range(2):
                        sl = slice(128 * g, 128 * (g + 1))
                        # transposed: stationary he, moving qew -> num^T[h,q];
                        # the tail then needs no PE transpose at all.
                        # den first on the last chunk: the sqrt chain is the
                        # tail's critical path
                        mm = [
                            (num_ps[g], he, qew),
                            (den_ps[g], he2, qe2),
                        ]
                        if m == MC - 1:
                            mm.reverse()
                        for ps, a, b in mm:
                            nc.tensor.matmul(
                                ps, a[:, m, sl], b[:, m, sl],
                                start=(m == 0), stop=(m == MC - 1),
                            )

                # transposed scores + softmax numerator while num/den PSUM is
                # still mapped; exp writes att^T directly as bf16 (the feat
                # GEMM stationary); both sqrts back-to-back keep ACT busy
                # while DVE runs the reciprocal/mask chain
                atts = []
                sds = []
                for g in range(2):
                    sd = tmp.tile([128, 128], F32, tag="sd")
                    nc.scalar.activation(sd[:], den_ps[g], ACT.Sqrt)
                    sds.append(sd)
                for g in range(2):
                    rd = tmp.tile([128, 128], F32, tag="rd")
                    nc.vector.reciprocal(rd[:], sds[g][:])
                    s = tmp.tile([128, 128], F32, tag="s")
                    nc.vector.tensor_mul(s[:], num_ps[g], rd[:])
                    nc.vector.tensor_add(s[:], s[:], mask)
                    atb = sb.tile([128, 128], BF16, name=f"atb{g}", tag=f"atb{g}")
                    nc.scalar.activation(atb[:], s[:], ACT.Exp)
                    atts.append(atb)

            # feat GEMMs; row-sums via a ones-vector matmul on the same
            # stationary att^T, renorm folded into the PSUM->SBUF copies
            with (
                tc.tile_pool(name="psf", bufs=2, space="PSUM") as psf,
                tc.tile_pool(name="psr", bufs=1, space="PSUM") as psr,
            ):
                for g in range(2):
                    atb = atts[g]
                    rsp = psr.tile([128, 1], F32, name=f"rs{g}", tag=f"rs{g}")
                    nc.tensor.matmul(rsp[:], atb[:], ones[:], start=True, stop=True)
                    rrs = sb.tile([128, 1], F32, name=f"rrs{g}", tag=f"rrs{g}")
                    nc.vector.reciprocal(rrs[:], rsp[:])
                    fsb = sb.tile([128, IN], BF16, name=f"fsb{g}", tag=f"fsb{g}")
                    for h in range(2):
                        # one 2-bank PSUM tile per half: two matmul groups
                        # (one full bank each), the two 512-wide scaled
                        # copies split across DVE and ACT, one DMA per half
                        fps = psf.tile([128, 1024], F32, tag="fps")
                        for c in range(2):
                            cs = slice(1024 * h + 512 * c, 1024 * h + 512 * (c + 1))
                            nc.tensor.matmul(
                                fps[:, 512 * c : 512 * (c + 1)], atb[:],
                                hn[:, g, cs], start=True, stop=True,
                            )
                        hs = slice(1024 * h, 1024 * (h + 1))
                        lo = slice(1024 * h, 1024 * h + 512)
                        hi = slice(1024 * h + 512, 1024 * (h + 1))
                        nc.vector.tensor_scalar_mul(
                            fsb[:, lo], fps[:, :512], rrs[:]
                        )
                        nc.scalar.activation(
                            fsb[:, hi], fps[:, 512:], ACT.Copy, scale=rrs[:]
                        )
                        nc.sync.dma_start(feat_d[g, :, hs], fsb[:, hs])

    _cheapen_drain_chain(nc)
    _split_multi_waits(nc)
    return nc


# ---------------------------------------------------------------------------
# Host side
# ---------------------------------------------------------------------------

_PROG_CACHE = {}


def _get_prog(fp8):
    if fp8 not in _PROG_CACHE:
        _PROG_CACHE[fp8] = build_program(fp8)
    return _PROG_CACHE[fp8]


def _pack_acts(x, fp8):
    """[BR, IN] -> [128, KC(,2), BR] with k_eff = 256k+128j+p (fp8) or
    128k+p (bf16); contiguous per partition."""
    xt = np.ascontiguousarray(x.T)  # [IN, BR]
    if fp8:
        a = xt.reshape(8, 2, 128, BR).transpose(2, 0, 1, 3)
        return np.ascontiguousarray(np.clip(a, -240, 240)).astype(
            ml_dtypes.float8_e4m3
        )
    a = xt.reshape(16, 128, BR).transpose(1, 0, 2)
    return np.ascontiguousarray(a).astype(ml_dtypes.bfloat16)


def _pack_w(Wy, Wg, fp8):
    """2x[IN, H] -> [MC, 128, 2, KC(,2), 128], scaled for fp8."""
    def one(W):
        if fp8:
            # [k8, j2, p128, m8, h128] -> [m, p, k, j, h]
            a = W.reshape(8, 2, 128, MC, 128).transpose(3, 2, 0, 1, 4)
            a = np.clip(a * WSCALE, -240, 240)
            return a.astype(ml_dtypes.float8_e4m3)
        a = W.reshape(16, 128, MC, 128).transpose(2, 1, 0, 3)
        return a.astype(ml_dtypes.bfloat16)

    y, g = one(Wy), one(Wg)
    return np.ascontiguousarray(np.stack([y, g], axis=2))


def _prep_shared(W_hy, b_hy, W_hg, b_hg, W_qy, b_qy, W_qg, b_qg, W_att, fp8):
    def bvec(b):
        return np.ascontiguousarray(b.reshape(MC, 128).T).astype(np.float32)

    # transposed block-diagonal causal mask, indexed [h_row, q_row]:
    # 0 where (same example AND h_round <= q_round), NEG elsewhere
    r = np.arange(128)
    same_ex = r[:, None] // 32 == r[None, :] // 32
    causal_t = (r[:, None] % 32) <= (r[None, :] % 32)
    mask_t = np.where(same_ex & causal_t, 0.0, NEG).astype(np.float32)

    cb = np.stack(
        [bvec(b_qy), bvec(b_qg), bvec(b_hy), bvec(b_hg), bvec(W_att)], axis=1
    )
    return {
        "wh": _pack_w(W_hy, W_hg, fp8),
        "wq": _pack_w(W_qy, W_qg, fp8),
        "cb": np.ascontiguousarray(cb),
        "mi": np.ascontiguousarray(mask_t),
    }


def kernel(
    hist, ques, W_hy, b_hy, W_hg, b_hg, W_qy, b_qy, W_qg, b_qg, W_att, b_att,
    mode="fp8", trace=False,
):
    from concourse.bass_utils import run_bass_kernel_spmd

    fp8 = mode == "fp8"
    hist = np.asarray(hist, np.float32)
    ques = np.asarray(ques, np.float32)
    nc = _get_prog(fp8)
    shared = _prep_shared(
        np.asarray(W_hy, np.float32), np.asarray(b_hy, np.float32),
        np.asarray(W_hg, np.float32), np.asarray(b_hg, np.float32),
        np.asarray(W_qy, np.float32), np.asarray(b_qy, np.float32),
        np.asarray(W_qg, np.float32), np.asarray(b_qg, np.float32),
        np.asarray(W_att, np.float32), fp8,
    )
    in_maps = []
    for c in range(NCORES):
        hs = hist[c * BL : (c + 1) * BL].reshape(BR, IN)
        qs = ques[c * BL : (c + 1) * BL].reshape(BR, IN)
        im = dict(shared)
        im["qt"] = _pack_acts(qs, fp8)
        im["ht"] = _pack_acts(hs, fp8)
        im["hn"] = np.ascontiguousarray(
            hs.reshape(2, 128, IN).transpose(1, 0, 2)
        ).astype(ml_dtypes.bfloat16)
        in_maps.append(im)

    res = run_bass_kernel_spmd(
        nc, in_maps, core_ids=list(range(NCORES)), trace=trace
    )
    feat = np.concatenate(
        [
            r["feat"].astype(np.float32).reshape(BL, R, IN)
            for r in res.results
        ],
        axis=0,
    )
    if trace:
        return feat, res
    return feat


# revision 56
# speedup vs baseline: 1.0237x; 1.0228x over previous
"""Trainium2 Bass kernel for nn_H_ATT (GatedTrans pair-attention block).

Math (per example):
  HE = tanh(hist@W_hy+b_hy) * lrelu(hist@W_hg+b_hg)      [R, H]
  QE = tanh(ques@W_qy+b_qy) * lrelu(ques@W_qg+b_qg)      [R, H]
  num[q,h]  = sum_k QE[q,k]*W_att[k]*HE[h,k]
  den[q,h]  = sqrt(sum_k QE[q,k]^2 * HE[h,k]^2)
  s = num / max(den, eps)          (b_att cancels in softmax)
  att = causal_softmax(s)          (softmax*tril/renorm == masked softmax)
  feat = att @ hist                 [R, 2H]

Sharding: pure data parallel, 8 examples per core on 8 NeuronCores.

The embedding GEMMs dominate both PE time and HBM traffic; they run in
fp8(e4m3) with DoubleRow (weights pre-scaled by 256 on the host, descale
fused into the activation's scale argument). The score/att/feat path stays
bf16/f32. All DRAM inputs are host-packed so every DMA line is contiguous
per partition.
"""

import numpy as np
import ml_dtypes

import bass_rust
import concourse.bass as bass
import concourse.mybir as mybir
import concourse.tile as tile
from concourse.vector_clock import ScopedClock

# ---------------------------------------------------------------------------
# Workaround: this walrus build accepts only ONE semaphore wait on an SP
# Drain, but TileContext's tail drain carries one wait per live semaphore.
# Split them across a chain of drains.
# ---------------------------------------------------------------------------


def _patched_drain_and_barrier(self, tick_clock, wait_clock):
    nc = self.nc
    drain_inst = nc.sync.drain()
    wait_clock.add_sem_waits(
        drain_inst.ins, ScopedClock({None: tick_clock.global_clock})
    )
    waits = list(drain_inst.ins.sync_info.on_wait)
    if len(waits) > 1:
        drain_inst.ins.sync_info = bass_rust.SyncInfo(
            on_wait=waits[:1], on_update=list(drain_inst.ins.sync_info.on_update)
        )
        for i in range(1, len(waits)):
            extra = nc.sync.drain()
            extra.ins.sync_info = bass_rust.SyncInfo(
                on_wait=waits[i : i + 1], on_update=[]
            )
    nc.all_engine_barrier()
    assert self.sems is not None
    popped = nc._tile_sem_poison_stack.pop()
    assert popped is self._sem_poison
    # clear_and_free_semaphores without the ~6us RANGE_CLEAR InstISA or
    # the Q7 dma_reset (redundant with the SP drain chain above). A
    # sem-wr-imm costs ~0.5us apiece, but sem-sub-imm is a cheap posted
    # op (same class as the barrier's inc/dec) — subtract each sem's
    # statically-known final value (the drain chain's wait values).
    finals = {w.id: (w.ant_name, w.wait_value) for w in waits}
    sems = list(self.sems.allocated().values())
    engines = [nc.sync, nc.scalar, nc.vector, nc.tensor, nc.gpsimd]
    for i, s in enumerate(sems):
        inst = engines[i % len(engines)].sem_inc(s, 0)
        u = inst.ins.sync_info.on_update[0]
        if u.id in finals:
            upd = bass_rust.SyncUpdate(
                sync_type="semaphore", id=u.id, ant_name=u.ant_name,
                update_mode="sem-sub-imm", update_value=finals[u.id][1],
            )
        else:
            upd = bass_rust.SyncUpdate(
                sync_type="semaphore", id=u.id, ant_name=u.ant_name,
                update_mode="sem-wr-imm", update_value=0,
            )
        inst.ins.sync_info = bass_rust.SyncInfo(
            on_wait=list(inst.ins.sync_info.on_wait), on_update=[upd]
        )
    sem_nums = [s.num for s in sems]
    nc._state.prepend_free_semaphores(sem_nums)
    for poison_set in nc._tile_sem_poison_stack:
        poison_set.update(sem_nums)
    nc.all_engine_barrier()


tile.TileContext._drain_and_barrier = _patched_drain_and_barrier


def _cheapen_drain_chain(nc):
    """The teardown emits a chain of SP Drains (one sem wait each; see
    _patched_drain_and_barrier). A Drain costs ~0.5us; an EventSemaphore
    wait is ~0.1us. Keep only the final Drain, turn the rest into waits."""
    bb = nc.m.functions[0].blocks[-1]
    run = []
    for i, inst in enumerate(bb.instructions):
        si = inst.sync_info
        if (
            isinstance(inst, mybir.InstDrain)
            and inst.engine == mybir.EngineType.SP
            and si is not None
            and len(si.on_wait) == 1
            and len(si.on_update) == 0
        ):
            run.append(i)
        else:
            break
    for i in run[:-1]:
        old = bb.instructions[i]
        nop = mybir.InstEventSemaphore(
            name=f"I-drainwait-{i}", ins=[], outs=[]
        )
        nop.engine = old.engine
        nop.sync_info = old.sync_info
        bb.instructions[i] = nop
    # the all_engine_barrier emits a bare Pool (Q7) Drain per round that
    # costs ~6us; this kernel issues no gpsimd DMAs, so there is nothing
    # to drain there — drop them
    bb.instructions[:] = [
        inst
        for inst in bb.instructions
        if not (
            isinstance(inst, mybir.InstDrain)
            and inst.engine == mybir.EngineType.Pool
            and (
                inst.sync_info is None
                or (
                    len(inst.sync_info.on_wait) == 0
                    and len(inst.sync_info.on_update) == 0
                )
            )
        )
    ]
    # drop the SECOND all_engine_barrier entirely: the sem-clear nops are
    # posted ops on each engine's own stream, so they complete before the
    # engine halts; the next execution's runtime start gate re-syncs.
    # (barrier-1 must stay — clears have to follow the DMA drain chain.)
    last_clear = None
    for i, inst in enumerate(bb.instructions):
        si = inst.sync_info
        if si is None:
            continue
        for u in si.on_update:
            if u.update_mode in ("sem-sub-imm", "sem-wr-imm") and (
                "barrier" not in u.ant_name
            ):
                last_clear = i
    if last_clear is not None:
        del bb.instructions[last_clear + 1 :]


def _thin_pe_sem_updates(nc):
    """Every matmul carries a PE sem-inc, and the sequencer sync pipe
    processes them at ~115ns each — a large backlog that delays the
    teardown. Only accumulation-group-final (stop=True) matmuls are ever
    waited on; strip the rest and renumber every wait/final on that sem."""
    insts = [i for f in nc.m.functions for bb in f.blocks for i in bb.instructions]
    pe_sem = None
    updaters = []
    for inst in insts:
        si = inst.sync_info
        if si is None:
            continue
        for u in si.on_update:
            if u.ant_name.startswith("PE_") and u.update_mode == "sem-inc":
                pe_sem = u.id
                updaters.append(inst)
    if pe_sem is None:
        return
    kept = [
        isinstance(inst, mybir.InstMatmult) and bool(inst.stop_tensor_calc)
        for inst in updaters
    ]
    # any wait pointing at a non-kept updater must keep that updater
    for inst in insts:
        si = inst.sync_info
        if si is None:
            continue
        for w in si.on_wait:
            if w.id == pe_sem and 0 < w.wait_value <= len(kept):
                kept[w.wait_value - 1] = True
    prefix = []
    c = 0
    for k in kept:
        c += k
        prefix.append(c)
    total = c
    # strip updates from non-kept matmuls
    for inst, k in zip(updaters, kept):
        if k:
            continue
        si = inst.sync_info
        inst.sync_info = bass_rust.SyncInfo(
            on_wait=list(si.on_wait),
            on_update=[u for u in si.on_update if u.id != pe_sem],
        )
    # renumber waits and the teardown's sem-sub final
    for inst in insts:
        si = inst.sync_info
        if si is None:
            continue
        dirty = False
        ws = []
        for w in si.on_wait:
            if w.id == pe_sem and 0 < w.wait_value <= len(kept):
                ws.append(
                    bass_rust.SyncWait(
                        sync_type=w.sync_type, id=w.id, ant_name=w.ant_name,
                        wait_mode=w.wait_mode,
                        wait_value=prefix[w.wait_value - 1],
                    )
                )
                dirty = True
            else:
                ws.append(w)
        us = []
        for u in si.on_update:
            if u.id == pe_sem and u.update_mode == "sem-sub-imm":
                us.append(
                    bass_rust.SyncUpdate(
                        sync_type=u.sync_type, id=u.id, ant_name=u.ant_name,
                        update_mode="sem-sub-imm", update_value=total,
                    )
                )
                dirty = True
            else:
                us.append(u)
        if dirty:
            inst.sync_info = bass_rust.SyncInfo(on_wait=ws, on_update=us)


def _split_multi_waits(nc):
    """This walrus build accepts at most one semaphore wait per instruction.
    Hoist extra waits onto standalone EventSemaphore instructions inserted
    just before the owning instruction in the same engine's stream."""
    uid = [0]
    for f in nc.m.functions:
        for bb in f.blocks:
            out = []
            for inst in bb.instructions:
                si = inst.sync_info
                if si is not None and len(si.on_wait) > 1:
                    waits = list(si.on_wait)
                    for w in waits[:-1]:
                        nop = mybir.InstEventSemaphore(
                            name=f"I-waitsplit-{uid[0]}", ins=[], outs=[]
                        )
                        uid[0] += 1
                        nop.engine = inst.engine
                        nop.sync_info = bass_rust.SyncInfo(
                            on_wait=[w], on_update=[]
                        )
                        out.append(nop)
                    inst.sync_info = bass_rust.SyncInfo(
                        on_wait=[waits[-1]], on_update=list(si.on_update)
                    )
                out.append(inst)
            bb.instructions[:] = out

# ---------------------------------------------------------------------------

B, R, H, IN = 64, 32, 1024, 2048
NCORES = 8
BL = B // NCORES  # examples per core
BR = BL * R  # 256 rows per core
MC = H // 128  # 8 h chunks
NEG = -1.0e30
WSCALE = 256.0  # fp8 weight pre-scale

F32 = mybir.dt.float32
BF16 = mybir.dt.bfloat16
E4 = mybir.dt.float8e4

ACT = mybir.ActivationFunctionType
DR = mybir.MatmulPerfMode.DoubleRow


def build_program(fp8=True):
    """Per-core Bass program. fp8: embedding GEMMs in e4m3 + DoubleRow;
    else bf16 regular matmuls. Everything downstream is identical."""
    xdt = E4 if fp8 else BF16
    KC = 8 if fp8 else 16  # contraction chunks ([128,2] pairs when fp8)
    kshape = [KC, 2, 128] if fp8 else [KC, 128]
    ascale = 1.0 / WSCALE if fp8 else 1.0

    nc = bass.Bass()
    # activations: [128, KC(,2), BR], partition-major contiguous
    qt_d = nc.dram_tensor("qt", [128] + kshape[:-1] + [BR], xdt, kind="ExternalInput")
    ht_d = nc.dram_tensor("ht", [128] + kshape[:-1] + [BR], xdt, kind="ExternalInput")
    # weights: [MC, 128, 2branch, KC(,2), 128]
    wh_d = nc.dram_tensor("wh", [MC, 128, 2] + kshape, xdt, kind="ExternalInput")
    wq_d = nc.dram_tensor("wq", [MC, 128, 2] + kshape, xdt, kind="ExternalInput")
    hn_d = nc.dram_tensor("hn", [128, 2, IN], BF16, kind="ExternalInput")
    # consts packed: [bqy, bqg, bhy, bhg, watt] along dim1
    cb_d = nc.dram_tensor("cb", [128, 5, MC], F32, kind="ExternalInput")
    # transposed causal/block mask [h, q]
    mi_d = nc.dram_tensor("mi", [128, 128], F32, kind="ExternalInput")
    feat_d = nc.dram_tensor("feat", [2, 128, IN], BF16, kind="ExternalOutput")

    with tile.TileContext(nc) as tc:
        with (
            tc.tile_pool(name="sb", bufs=1) as sb,
            tc.tile_pool(name="wts", bufs=8) as wts,
            tc.tile_pool(name="tmp", bufs=3) as tmp,
        ):
            # acts + consts on the ACT hwdge queue (independent of the
            # weight queue so weight-buffer waits never delay them).
            # qt split in two tiles so the first matmuls start earlier.
            KH = KC // 2
            qta = sb.tile([128, KH] + kshape[1:-1] + [BR], xdt, tag="qta")
            nc.scalar.dma_start(qta[:], qt_d[:, :KH])
            qtb = sb.tile([128, KH] + kshape[1:-1] + [BR], xdt, tag="qtb")
            nc.scalar.dma_start(qtb[:], qt_d[:, KH:])

            def qt(k):
                return qta[:, k] if k < KH else qtb[:, k - KH]

            # tiny consts next (first activations need the biases), then
            # the bulk hist loads (not needed until the hist phase / tail)
            cb = sb.tile([128, 5, MC], F32, tag="cb")
            nc.scalar.dma_start(cb[:], cb_d[:])
            bsb = {
                "bqy": cb[:, 0], "bqg": cb[:, 1],
                "bhy": cb[:, 2], "bhg": cb[:, 3],
            }
            watt = cb[:, 4]

            ht_t = sb.tile([128] + kshape[:-1] + [BR], xdt, tag="ht")
            nc.scalar.dma_start(ht_t[:], ht_d[:])

            def ht(k):
                return ht_t[:, k]

            mask_t = sb.tile([128, 128], F32, tag="mask")
            mask = mask_t[:]
            hn = sb.tile([128, 2, IN], BF16, tag="hn")
            ones = sb.tile([128, 1], BF16, tag="ones")
            nc.vector.memset(ones[:], 1.0)

            he = sb.tile([128, MC, BR], BF16, tag="he")
            he2 = sb.tile([128, MC, BR], BF16, tag="he2")
            qew = sb.tile([128, MC, BR], BF16, tag="qew")
            qe2 = sb.tile([128, MC, BR], BF16, tag="qe2")

            with (
                tc.tile_pool(name="pse", bufs=2, space="PSUM") as pse,
                tc.tile_pool(name="psnd", bufs=1, space="PSUM") as psnd,
            ):
                num_ps = [
                    psnd.tile([128, 128], F32, name=f"num{g}", tag=f"num{g}")
                    for g in range(2)
                ]
                den_ps = [
                    psnd.tile([128, 128], F32, name=f"den{g}", tag=f"den{g}")
                    for g in range(2)
                ]
                num_ps = [t[:] for t in num_ps]
                den_ps = [t[:] for t in den_ps]

                # weight DMAs pre-issued on the sync queue: wh0 interleaved
                # before the ques tail so the hist phase isn't gated on it
                # (the queue delivers ~one 512KB tile per 1.6us, which only
                # just keeps pace with PE's ~1.75us per m-chunk)
                worder = (
                    [("q", m) for m in range(6)]
                    + [("h", 0), ("q", 6), ("q", 7)]
                    + [("h", m) for m in range(1, MC)]
                )
                wtiles = {}
                for i, (t, m) in enumerate(worder):
                    wt = wts.tile([128, 2] + kshape, xdt, tag="wt")
                    src_d = (wq_d if t == "q" else wh_d)[m]
                    if i == 0:
                        # split the very first weight tile by branch so the
                        # first psy matmuls start half a transfer earlier
                        nc.sync.dma_start(wt[:, 0], src_d[:, 0])
                        nc.sync.dma_start(wt[:, 1], src_d[:, 1])
                    else:
                        nc.sync.dma_start(wt[:], src_d)
                    wtiles[(t, m)] = wt
                # tail-only loads at the back of the weight queue: keeps
                # their triggers off the ACT sequencer entirely
                nc.sync.dma_start(mask_t[:], mi_d[:])
                nc.sync.dma_start(hn[:], hn_d[:])

                def gated(xt, tkey, by, bg, m):
                    """Embedding GEMM pair + activations; returns ty, tg f32."""
                    wt = wtiles[(tkey, m)]
                    psy = pse.tile([128, BR], F32, tag="psy")
                    psg = pse.tile([128, BR], F32, tag="psg")
                    for br, ps in ((0, psy), (1, psg)):
                        for k in range(KC):
                            if fp8:
                                nc.tensor.matmul(
                                    ps[:], wt[:, br, k], xt(k),
                                    start=(k == 0), stop=(k == KC - 1),
                                    perf_mode=DR,
                                )
                            else:
                                nc.tensor.matmul(
                                    ps[:], wt[:, br, k], xt(k),
                                    start=(k == 0), stop=(k == KC - 1),
                                )
                    ty = tmp.tile([128, BR], F32, tag="ty")
                    nc.scalar.activation(
                        ty[:], psy[:], ACT.Tanh,
                        bias=by[:, m : m + 1], scale=ascale,
                    )
                    tg = tmp.tile([128, BR], F32, tag="tg")
                    nc.scalar.activation(
                        tg[:], psg[:], ACT.Lrelu,
                        bias=bg[:, m : m + 1], scale=ascale, alpha=0.01,
                    )
                    return ty, tg

                # ques embeddings
                for m in range(MC):
                    ty, tg = gated(qt, "q", bsb["bqy"], bsb["bqg"], m)
                    nc.vector.scalar_tensor_tensor(
                        qew[:, m, :], ty[:], watt[:, m : m + 1], tg[:],
                        op0=mybir.AluOpType.mult, op1=mybir.AluOpType.mult,
                    )
                    qe = tmp.tile([128, BR], F32, tag="qe")
                    nc.vector.tensor_mul(qe[:], ty[:], tg[:])
                    nc.gpsimd.tensor_mul(qe2[:, m, :], qe[:], qe[:])

                # hist embeddings + num/den accumulation per chunk
                for m in range(MC):
                    ty, tg = gated(ht, "h", bsb["bhy"], bsb["bhg"], m)
                    nc.vector.tensor_mul(he[:, m, :], ty[:], tg[:])
                    nc.gpsimd.tensor_mul(he2[:, m, :], he[:, m, :], he[:, m, :])
                    for g in range(2):
                        sl = slice(128 * g, 128 * (g + 1))
                        # transposed: stationary he, moving qew -> num^T[h,q];
                        # the tail then needs no PE transpose at all.
                        # den first on the last chunk: the sqrt chain is the
                        # tail's critical path
                        mm = [
                            (num_ps[g], he, qew),
                            (den_ps[g], he2, qe2),
                        ]
                        if m == MC - 1:
                            mm.reverse()
                        for ps, a, b in mm:
                            nc.tensor.matmul(
                                ps, a[:, m, sl], b[:, m, sl],
                                start=(m == 0), stop=(m == MC - 1),
                            )

                # transposed scores + softmax numerator while num/den PSUM is
                # still mapped; exp writes att^T directly as bf16 (the feat
                # GEMM stationary); both sqrts back-to-back keep ACT busy
                # while DVE runs the reciprocal/mask chain
                atts = []
                sds = []
                for g in range(2):
                    sd = tmp.tile([128, 128], F32, tag="sd")
                    nc.scalar.activation(sd[:], den_ps[g], ACT.Sqrt)
                    sds.append(sd)
                for g in range(2):
                    rd = tmp.tile([128, 128], F32, tag="rd")
                    nc.vector.reciprocal(rd[:], sds[g][:])
                    s = tmp.tile([128, 128], F32, tag="s")
                    nc.vector.tensor_mul(s[:], num_ps[g], rd[:])
                    nc.vector.tensor_add(s[:], s[:], mask)
                    atb = sb.tile([128, 128], BF16, name=f"atb{g}", tag=f"atb{g}")
                    nc.scalar.activation(atb[:], s[:], ACT.Exp)
                    atts.append(atb)

            # feat GEMMs; row-sums via a ones-vector matmul on the same
            # stationary att^T, renorm folded into the PSUM->SBUF copies
            with (
                tc.tile_pool(name="psf", bufs=3, space="PSUM") as psf,
                tc.tile_pool(name="psr", bufs=1, space="PSUM") as psr,
            ):
                for g in range(2):
                    atb = atts[g]
                    rsp = psr.tile([128, 1], F32, name=f"rs{g}", tag=f"rs{g}")
                    nc.tensor.matmul(rsp[:], atb[:], ones[:], start=True, stop=True)
                    rrs = sb.tile([128, 1], F32, name=f"rrs{g}", tag=f"rrs{g}")
                    nc.vector.reciprocal(rrs[:], rsp[:])
                    fsb = sb.tile([128, IN], BF16, name=f"fsb{g}", tag=f"fsb{g}")
                    for h in range(2):
                        # one 2-bank PSUM tile per half: two matmul groups
                        # (one full bank each), the two 512-wide scaled
                        # copies split across DVE and ACT, one DMA per half
                        fps = psf.tile([128, 1024], F32, tag="fps")
                        for c in range(2):
                            cs = slice(1024 * h + 512 * c, 1024 * h + 512 * (c + 1))
                            nc.tensor.matmul(
                                fps[:, 512 * c : 512 * (c + 1)], atb[:],
                                hn[:, g, cs], start=True, stop=True,
                            )
                        hs = slice(1024 * h, 1024 * (h + 1))
                        lo = slice(1024 * h, 1024 * h + 512)
                        hi = slice(1024 * h + 512, 1024 * (h + 1))
                        nc.vector.tensor_scalar_mul(
                            fsb[:, lo], fps[:, :512], rrs[:]
                        )
                        nc.scalar.activation(
                            fsb[:, hi], fps[:, 512:], ACT.Copy, scale=rrs[:]
                        )
                        nc.sync.dma_start(feat_d[g, :, hs], fsb[:, hs])

    _cheapen_drain_chain(nc)
    _split_multi_waits(nc)
    return nc


# ---------------------------------------------------------------------------
# Host side
# ---------------------------------------------------------------------------

_PROG_CACHE = {}


def _get_prog(fp8):
    if fp8 not in _PROG_CACHE:
        _PROG_CACHE[fp8] = build_program(fp8)
    return _PROG_CACHE[fp8]


def _pack_acts(x, fp8):
    """[BR, IN] -> [128, KC(,2), BR] with k_eff = 256k+128j+p (fp8) or
    128k+p (bf16); contiguous per partition."""
    xt = np.ascontiguousarray(x.T)  # [IN, BR]
    if fp8:
        a = xt.reshape(8, 2, 128, BR).transpose(2, 0, 1, 3)
        return np.ascontiguousarray(np.clip(a, -240, 240)).astype(
            ml_dtypes.float8_e4m3
        )
    a = xt.reshape(16, 128, BR).transpose(1, 0, 2)
    return np.ascontiguousarray(a).astype(ml_dtypes.bfloat16)


def _pack_w(Wy, Wg, fp8):
    """2x[IN, H] -> [MC, 128, 2, KC(,2), 128], scaled for fp8."""
    def one(W):
        if fp8:
            # [k8, j2, p128, m8, h128] -> [m, p, k, j, h]
            a = W.reshape(8, 2, 128, MC, 128).transpose(3, 2, 0, 1, 4)
            a = np.clip(a * WSCALE, -240, 240)
            return a.astype(ml_dtypes.float8_e4m3)
        a = W.reshape(16, 128, MC, 128).transpose(2, 1, 0, 3)
        return a.astype(ml_dtypes.bfloat16)

    y, g = one(Wy), one(Wg)
    return np.ascontiguousarray(np.stack([y, g], axis=2))


def _prep_shared(W_hy, b_hy, W_hg, b_hg, W_qy, b_qy, W_qg, b_qg, W_att, fp8):
    def bvec(b):
        return np.ascontiguousarray(b.reshape(MC, 128).T).astype(np.float32)

    # transposed block-diagonal causal mask, indexed [h_row, q_row]:
    # 0 where (same example AND h_round <= q_round), NEG elsewhere
    r = np.arange(128)
    same_ex = r[:, None] // 32 == r[None, :] // 32
    causal_t = (r[:, None] % 32) <= (r[None, :] % 32)
    mask_t = np.where(same_ex & causal_t, 0.0, NEG).astype(np.float32)

    cb = np.stack(
        [bvec(b_qy), bvec(b_qg), bvec(b_hy), bvec(b_hg), bvec(W_att)], axis=1
    )
    return {
        "wh": _pack_w(W_hy, W_hg, fp8),
        "wq": _pack_w(W_qy, W_qg, fp8),
        "cb": np.ascontiguousarray(cb),
        "mi": np.ascontiguousarray(mask_t),
    }


def kernel(
    hist, ques, W_hy, b_hy, W_hg, b_hg, W_qy, b_qy, W_qg, b_qg, W_att, b_att,
    mode="fp8", trace=False,
):
    from concourse.bass_utils import run_bass_kernel_spmd

    fp8 = mode == "fp8"
    hist = np.asarray(hist, np.float32)
    ques = np.asarray(ques, np.float32)
    nc = _get_prog(fp8)
    shared = _prep_shared(
        np.asarray(W_hy, np.float32), np.asarray(b_hy, np.float32),
        np.asarray(W_hg, np.float32), np.asarray(b_hg, np.float32),
        np.asarray(W_qy, np.float32), np.asarray(b_qy, np.float32),
        np.asarray(W_qg, np.float32), np.asarray(b_qg, np.float32),
        np.asarray(W_att, np.float32), fp8,
    )
    in_maps = []
    for c in range(NCORES):
        hs = hist[c * BL : (c + 1) * BL].reshape(BR, IN)
        qs = ques[c * BL : (c + 1) * BL].reshape(BR, IN)
        im = dict(shared)
        im["qt"] = _pack_acts(qs, fp8)
        im["ht"] = _pack_acts(hs, fp8)
        im["hn"] = np.ascontiguousarray(
            hs.reshape(2, 128, IN).transpose(1, 0, 2)
        ).astype(ml_dtypes.bfloat16)
        in_maps.append(im)

    res = run_bass_kernel_spmd(
        nc, in_maps, core_ids=list(range(NCORES)), trace=trace
    )
    feat = np.concatenate(
        [
            r["feat"].astype(np.float32).reshape(BL, R, IN)
            for r in res.results
        ],
        axis=0,
    )
    if trace:
        return feat, res
    return feat
